# revision 1
# baseline (speedup 1.0000x reference)
"""Causal self-attention kernel for Trainium2, 8 NeuronCores, data-parallel over batch.

Problem: B=4096 independent attentions, T=64, DIM=128, 4 heads of 32; y = proj(attn(x)).
k_in / v_in inputs are unused by the module (overwritten internally) -> never shipped.

Strategy (per core: 512 batches = 32768 tokens, processed in 64 "mega-tiles" of 512 tokens):
  - x loaded natural [tok, D], PE-transposed to x^T [D, tok].
  - q^T = (Wq/sqrt(hs))^T x^T, k^T = Wk^T x^T (weights stationary), v = x@Wv natural.
  - scores computed TRANSPOSED [keys, (pair,head,query)]; causal mask (-80) pre-seeded
    into PSUM by an identity matmul, score matmuls accumulate on top.
  - one exp() on ACT -> attn_u^T in SBUF. Denominators via block-indicator matmul
    (sums over key-partitions), reciprocal, then broadcast back over the 64 key
    partitions with a tiny-K matmul; one DVE multiply normalizes.
  - y^T = v^T-weighted attention via v-natural slices as stationary operand,
    proj back to natural [tok, D] layout, bias, DMA out.
  Bias algebra: k-bias dropped (constant per softmax column -> invariant);
  v-bias folded into proj bias on host (attn rows sum to 1); q-bias is a
  per-partition add; proj(+v) bias is one DVE tensor add.
"""

import sys

for _p in ("/opt/trn_rl_repo", "/root/.axon_site/_ro/trn_rl_repo"):
    if _p not in sys.path:
        sys.path.insert(0, _p)

from contextlib import ExitStack

import numpy as np

import concourse.bass as bass
import concourse.tile as tile
from concourse import bacc
from concourse import mybir
from concourse.bass_utils import run_bass_kernel_spmd

F32 = mybir.dt.float32

B, T, D, H, HS = 4096, 64, 128, 4, 32
NCORES = 8
BC = B // NCORES            # 512 batches per core
TOK = BC * T                # 32768 tokens per core
MEGA = 512                  # tokens per mega-tile (8 batches = 4 batch-pairs)
NMEGA = TOK // MEGA         # 64
SCALE = 1.0 / float(np.sqrt(HS))
NEG = -80.0                 # additive causal mask (exp(-80) ~ 1.8e-35, harmless)

_CACHE = {}
LAST_RESULT = None


def _host_consts(W_attn, b_attn, W_proj, b_proj):
    """Precompute all constant tiles on host (float32 numpy)."""
    Wq = np.ascontiguousarray(W_attn[:, 0:128] * SCALE)          # [128,128]
    Wk = np.ascontiguousarray(W_attn[:, 128:256])                # [128,128]
    Wv = np.ascontiguousarray(W_attn[:, 256:384])                # [128,128]
    bqs = b_attn[0:128] * SCALE
    # bq2[:, j] = q-bias for head-pair j, stored on partitions 0..63
    bq2 = np.stack([bqs[0:64], bqs[64:128]], axis=1)             # [64,2]
    bv = b_attn[256:384]
    bp_eff = b_proj + bv @ W_proj                                # [128]
    biasP = np.tile(bp_eff.reshape(1, 128), (128, 4)).reshape(128, 512)
    # biasP[p, c*128+f] = bp_eff[f]
    biasP = np.ascontiguousarray(
        np.broadcast_to(bp_eff.reshape(1, 1, 128), (128, 4, 128)).reshape(128, 512)
    )
    ident = np.eye(128, dtype=np.float32)
    # causal mask, transposed-scores layout: [row=b*64+kk, col=pp*256+h*64+qq]
    kk = np.arange(64).reshape(64, 1)
    qq = np.arange(64).reshape(1, 64)
    m0 = np.where(kk <= qq, 0.0, NEG).astype(np.float32)         # [64,64]
    maskT = np.tile(np.tile(m0, (2, 1)), (1, 16))                # [128, 1024]
    # sums lhsT: col 0/1 = indicator of key half; cols 2..31 zero (pads the
    # output to a full 32-partition block so no PSUM row is left undefined)
    blockind = np.zeros((128, 32), dtype=np.float32)
    blockind[0:64, 0] = 1.0
    blockind[64:128, 1] = 1.0
    # recB lhsT: rows 32p+b carry indicator of output half b
    blockT2 = np.zeros((128, 128), dtype=np.float32)
    for p in range(4):
        blockT2[32 * p + 0, 0:64] = 1.0
        blockT2[32 * p + 1, 64:128] = 1.0
    parts = [
        ("wq", Wq.astype(np.float32)),
        ("wk", Wk.astype(np.float32)),
        ("wv", Wv.astype(np.float32)),
        ("wp", np.ascontiguousarray(W_proj).astype(np.float32)),
        ("bq2", np.ascontiguousarray(np.pad(bq2, ((0, 64), (0, 0)))).astype(np.float32)),
        ("wps", np.ascontiguousarray(
            np.pad(np.concatenate([W_proj[0:64], W_proj[64:128]], axis=1),
                   ((0, 64), (0, 0)))).astype(np.float32)),
        ("biasP", biasP.astype(np.float32)),
        ("ident", ident),
        ("maskT", np.ascontiguousarray(maskT).astype(np.float32)),
        ("blockind", blockind),
        ("blockT2", blockT2),
    ]
    packed = np.concatenate([a for _, a in parts], axis=1).astype(np.float32)
    offsets = {}
    off = 0
    for name, a in parts:
        offsets[name] = (off, a.shape[1])
        off += a.shape[1]
    return packed, offsets


def _build_program(const_offsets, const_cols, ntok=TOK, stage=9):
    nmega = ntok // MEGA
    nc = bacc.Bacc()
    x_p = nc.declare_dram_parameter("x", [ntok, D], F32, isOutput=False)
    y_p = nc.declare_dram_parameter("y", [ntok, D], F32, isOutput=True)
    c_p = nc.declare_dram_parameter("consts", [128, const_cols], F32, isOutput=False)

    # token index: t = m*512 + c*128 + p  (c = chunk / batch-pair, p = partition)
    x_v = x_p.rearrange("(m c p) d -> m p c d", c=4, p=128)
    y_v = y_p.rearrange("(m c p) d -> m p c d", c=4, p=128)

    # scores^T column map: col = 512*(h%2) + 128*pp + 64*(h//2) + q
    # (bank = h%2, so every PSUM bank is written by matmuls whose stationary
    # operand sits at ONE partition base -- concurrent different-row-strip
    # writes to a shared bank are a hardware fault)
    def scol(pp, h):
        return 512 * (h % 2) + 128 * pp + 64 * (h // 2)

    with tile.TileContext(nc) as tc, ExitStack() as ctx:
        cpool = ctx.enter_context(tc.tile_pool(name="consts", bufs=1))
        sb = ctx.enter_context(tc.tile_pool(name="sb", bufs=2))
        psA = ctx.enter_context(tc.tile_pool(name="psA", bufs=4, space="PSUM"))
        psB = ctx.enter_context(tc.tile_pool(name="psB", bufs=2, space="PSUM"))

        call = cpool.tile([128, const_cols], F32, tag="c_all")
        nc.sync.dma_start(out=call[:], in_=c_p[:])
        ct = {
            name: call[:, off: off + w] for name, (off, w) in const_offsets.items()
        }
        Copy = mybir.ActivationFunctionType.Copy

        for m in range(nmega):
            # ---- load x natural [p, c, d]
            x_nat = sb.tile([128, 4, 128], F32, tag="x_nat")
            nc.sync.dma_start(out=x_nat[:], in_=x_v[m])

            # ---- x^T via PE transpose (4 chunks)
            xT_ps = psA.tile([128, 512], F32, tag="mm512")
            for c in range(4):
                nc.tensor.transpose(
                    xT_ps[:, c * 128:(c + 1) * 128], x_nat[:, c, :], ct["ident"]
                )
            xT = sb.tile([128, 512], F32, tag="xT")
            nc.scalar.activation(xT[:], xT_ps[:], Copy)
            if stage <= 1:
                nc.sync.dma_start(out=y_v[m], in_=xT[:].rearrange("p (c d) -> p c d", c=4))
                continue

            # ---- q^T, k^T as head-pair tiles at partition base 0 (base 96 is
            # unreachable without tile_position, which faults this toolchain)
            qk = []
            for j, w in ((0, "wq"), (1, "wq"), (0, "wk"), (1, "wk")):
                p = psA.tile([64, 512], F32, tag="mm512")
                nc.tensor.matmul(
                    p[:], ct[w][:, j * 64:(j + 1) * 64], xT[:],
                    start=True, stop=True,
                )
                qk.append(p)
            qaps, qbps, kaps, kbps = qk
            qtA = sb.tile([64, 512], F32, tag="qtA")
            nc.vector.tensor_scalar_add(qtA[:], qaps[:], ct["bq2"][0:64, 0:1])
            qtB = sb.tile([64, 512], F32, tag="qtB")
            nc.vector.tensor_scalar_add(qtB[:], qbps[:], ct["bq2"][0:64, 1:2])
            ktA = sb.tile([64, 512], F32, tag="ktA")
            nc.scalar.activation(ktA[:], kaps[:], Copy)
            ktB = sb.tile([64, 512], F32, tag="ktB")
            nc.scalar.activation(ktB[:], kbps[:], Copy)

            # ---- v natural [tok, feat]
            v_ps = psA.tile([128, 512], F32, tag="mm512")
            for c in range(4):
                nc.tensor.matmul(
                    v_ps[:, c * 128:(c + 1) * 128],
                    xT[:, c * 128:(c + 1) * 128],
                    ct["wv"],
                    start=True,
                    stop=True,
                )
            v_s = sb.tile([128, 512], F32, tag="v_s")
            nc.scalar.activation(v_s[:], v_ps[:], Copy)

            # ---- scores^T with pre-seeded causal mask
            sc_ps = psB.tile([128, 1024], F32, tag="mm1024")
            for half in range(2):
                nc.tensor.matmul(
                    sc_ps[:, half * 512:(half + 1) * 512],
                    ct["ident"],
                    ct["maskT"][:, half * 512:(half + 1) * 512],
                    start=True,
                    stop=False,
                    skip_group_check=True,
                )
            for h in range(4):
                qt = (qtA, qtB)[h // 2]
                kt = (ktA, ktB)[h // 2]
                r0 = 32 * (h % 2)
                for pp in range(4):
                    for b in range(2):
                        bb = pp * 2 + b
                        c0 = scol(pp, h)
                        nc.tensor.matmul(
                            sc_ps[b * 64:(b + 1) * 64, c0:c0 + 64],
                            kt[r0:r0 + 32, bb * 64:(bb + 1) * 64],
                            qt[r0:r0 + 32, bb * 64:(bb + 1) * 64],
                            start=False,
                            stop=(h == 3 and pp == 3 and b == 1),
                            skip_group_check=True,
                        )

            # ---- softmax (transposed layout)
            attn_u = sb.tile([128, 1024], F32, tag="attn_u")
            nc.scalar.activation(attn_u[:], sc_ps[:], mybir.ActivationFunctionType.Exp)
            if stage <= 3:
                nc.sync.dma_start(out=y_v[m], in_=attn_u[:, 0:512].rearrange("p (c d) -> p c d", c=4))
                continue

            # column sums (over keys = partitions) via indicator matmul;
            # all pairs land at partition base 0
            su_ps = psB.tile([32, 1024], F32, tag="mm1024")
            for pp in range(4):
                nc.tensor.matmul(
                    su_ps[0:32, 256 * pp:256 * (pp + 1)],
                    ct["blockind"][:],
                    attn_u[:].rearrange(
                        "p (hr pp rest) -> p hr pp rest", hr=2, pp=4
                    )[:, :, pp, :],
                    start=True,
                    stop=True,
                )
            rec = sb.tile([32, 1024], F32, tag="rec")
            nc.vector.reciprocal(rec[:], su_ps[:])

            rb_ps = psB.tile([128, 1024], F32, tag="mm1024")
            for hr in range(2):
                for pp in range(4):
                    nc.tensor.matmul(
                        rb_ps[:, 512 * hr + 128 * pp: 512 * hr + 128 * (pp + 1)],
                        ct["blockT2"][0:2, :],
                        rec[0:2, 256 * pp + 128 * hr: 256 * pp + 128 * (hr + 1)],
                        start=True,
                        stop=True,
                    )
            attn_n = sb.tile([128, 1024], F32, tag="attn_n")
            nc.vector.tensor_mul(attn_n[:], attn_u[:], rb_ps[:])
            if stage <= 5:
                nc.sync.dma_start(out=y_v[m], in_=attn_n[:, 0:512].rearrange("p (c d) -> p c d", c=4))
                continue

            # ---- y^T: four tiles split by (batch-half, head-pair) so every
            # PSUM bank sees one stationary partition base and out bases
            # stay in {0,32}
            yts = {}
            for b in (0, 1):
                for X in (0, 1):
                    yt = psA.tile([64, 256], F32, tag="mm512")
                    yts[(b, X)] = yt
                    for pp in range(4):
                        for hr in range(2):
                            h = 2 * X + hr
                            c0 = scol(pp, h)
                            nc.tensor.matmul(
                                yt[hr * 32:(hr + 1) * 32, pp * 64:(pp + 1) * 64],
                                v_s[b * 64:(b + 1) * 64,
                                    pp * 128 + h * 32: pp * 128 + (h + 1) * 32],
                                attn_n[b * 64:(b + 1) * 64, c0:c0 + 64],
                                start=True,
                                stop=True,
                            )
            # reassemble y^T head-pair tiles [64, tok]: token col = 128*pp + 64*b + q
            yTs = []
            for X in (0, 1):
                yTX = sb.tile([64, 512], F32, tag=f"yT{X}")
                yTs.append(yTX)
                yTX_v = yTX[:].rearrange("f (pp b q) -> f pp b q", pp=4, b=2)
                for b in (0, 1):
                    src_v = yts[(b, X)][:].rearrange("f (pp q) -> f pp q", pp=4)
                    if b == 0:
                        nc.scalar.activation(yTX_v[:, :, b, :], src_v, Copy)
                    else:
                        nc.vector.tensor_copy(yTX_v[:, :, b, :], src_v)

            # ---- projection (two K=64 accumulating matmuls per chunk) + bias
            yf_ps = psA.tile([128, 512], F32, tag="mm512")
            for c in range(4):
                for X in (0, 1):
                    nc.tensor.matmul(
                        yf_ps[:, c * 128:(c + 1) * 128],
                        yTs[X][:, c * 128:(c + 1) * 128],
                        ct["wps"][0:64, X * 128:(X + 1) * 128],
                        start=(X == 0),
                        stop=(X == 1),
                    )
            y_out = sb.tile([128, 512], F32, tag="y_out")
            nc.vector.tensor_add(y_out[:], yf_ps[:], ct["biasP"])

            nc.sync.dma_start(out=y_v[m], in_=y_out[:].rearrange("p (c d) -> p c d", c=4))
    nc.compile()
    return nc


def kernel(x, k_in, v_in, W_attn, b_attn, W_proj, b_proj):
    x = np.asarray(x, dtype=np.float32)
    packed, offsets = _host_consts(
        np.asarray(W_attn, dtype=np.float32),
        np.asarray(b_attn, dtype=np.float32),
        np.asarray(W_proj, dtype=np.float32),
        np.asarray(b_proj, dtype=np.float32),
    )
    key = "prog"
    if key not in _CACHE:
        _CACHE[key] = _build_program(offsets, packed.shape[1])
    nc = _CACHE[key]

    in_maps = []
    for i in range(NCORES):
        shard = np.ascontiguousarray(
            x[i * BC:(i + 1) * BC].reshape(TOK, D)
        )
        in_maps.append({"x": shard, "consts": packed})

    res = run_bass_kernel_spmd(nc, in_maps, list(range(NCORES)))
    global LAST_RESULT
    LAST_RESULT = res
    outs = [res.results[i]["y"].reshape(BC, T, D) for i in range(NCORES)]
    return np.concatenate(outs, axis=0)


if __name__ == "__main__":
    rng = np.random.default_rng(0)
    xs = rng.standard_normal((B, T, D), dtype=np.float32)
    Wa = rng.standard_normal((D, 3 * D), dtype=np.float32) / np.sqrt(D)
    ba = rng.standard_normal(3 * D, dtype=np.float32) * 0.01
    Wp = rng.standard_normal((D, D), dtype=np.float32) / np.sqrt(D)
    bp = rng.standard_normal(D, dtype=np.float32) * 0.01
    out = kernel(xs, None, None, Wa, ba, Wp, bp)
    print(out.shape, out.dtype)



# revision 12
# speedup vs baseline: 1.6750x; 1.6750x over previous
"""Causal self-attention kernel for Trainium2, 8 NeuronCores, data-parallel over batch.

Problem: B=4096 independent attentions, T=64, DIM=128, 4 heads of 32; y = proj(attn(x)).
k_in / v_in inputs are unused by the module (overwritten internally) -> never shipped.

v1 "chunk-dense bf16" design (per core: 512 batches = 32768 tokens, 64 mega-tiles
of 512 tokens = 4 chunks of 128 tokens = 2 batches each):
  - All matmuls bf16 (fp32 matmuls run as 2 half-speed passes; bf16 is 4x).
  - x loaded natural, PE-transposed (fp32), cast to bf16 in the PSUM->SBUF copy.
  - q^T/k^T as head-pair tiles [64,512] (partition base 96 is rejected by the
    toolchain, so features of head 3 must sit at base 32 of a pair tile).
    q-bias added in the PSUM->SBUF copy (per-partition tensor_scalar); k-bias
    dropped (softmax invariant); v-bias folded into proj bias on host.
  - scores chunk-dense TRANSPOSED: per (head h, chunk c) one [32,128]x[32,128]
    -> [128,128] matmul; the 2 batches inside a chunk are separated by the mask
    seed (-80 on cross-batch and causal-invalid pairs), so exp() kills them.
    PSUM bank = head; each bank's accumulation group is (seed + 4 scorers).
  - softmax denominators: 4 accumulating indicator matmuls -> su[4,512]
    (row = head), reciprocal_approx_fast (18 bits, ~5x faster than
    nc.vector.reciprocal), broadcast over each head's 32 features with fp32r
    matmuls (fp32r runs 1 cyc/row when out free >= 256 -- no bf16 cast needed).
  - attn@v with UNNORMALIZED attn (normalization commutes with the key-sum):
    per (h, c) one [128,32]x[128,128] matmul -> ytX[32*(h%2)+d, tok] for
    X=h//2; each [64,512] ytX is normalized by rec in the PSUM->SBUF copy.
  - proj natural: per chunk two accumulating K=64 matmuls (yTA/yTB stationary),
    bias via gpsimd tensor_add; proj of mega m-1 is emitted inside mega m
    (software pipelining) so the PE never waits on the DVE normalize.
  - elementwise ops spread across ACT (xT copy, exp), DVE (q-bias, recip,
    normalize), Pool/GpSimd (k/v copies, y bias add) to keep all engines busy.
"""

import sys

for _p in ("/opt/trn_rl_repo", "/root/.axon_site/_ro/trn_rl_repo"):
    if _p not in sys.path:
        sys.path.insert(0, _p)

from contextlib import ExitStack

import numpy as np
import ml_dtypes

import concourse.bass as bass
import concourse.tile as tile
from concourse import bacc
from concourse import mybir
from concourse.bass_utils import run_bass_kernel_spmd

F32 = mybir.dt.float32
F32R = mybir.dt.float32r
BF16 = mybir.dt.bfloat16
NP_BF16 = ml_dtypes.bfloat16

B, T, D, H, HS = 4096, 64, 128, 4, 32
NCORES = 8
BC = B // NCORES            # 512 batches per core
TOK = BC * T                # 32768 tokens per core
MEGA = 512                  # tokens per mega-tile (4 chunks of 128 = 8 batches)
NMEGA = TOK // MEGA         # 64
SCALE = 1.0 / float(np.sqrt(HS))
NEG = -80.0                 # additive causal mask (exp(-80) ~ 1.8e-35, harmless)

_CACHE = {}
LAST_RESULT = None


def _host_consts(W_attn, b_attn, W_proj, b_proj):
    """Precompute constant tiles on host: one fp32 pack and one bf16 pack."""
    Wq = np.ascontiguousarray(W_attn[:, 0:128] * SCALE)          # [128,128]
    Wk = np.ascontiguousarray(W_attn[:, 128:256])
    Wv = np.ascontiguousarray(W_attn[:, 256:384])
    bqs = (b_attn[0:128] * SCALE).reshape(128, 1)                # [128,1]
    bv = b_attn[256:384]
    bp_eff = b_proj + bv @ W_proj                                # [128]
    biasP = np.ascontiguousarray(
        np.broadcast_to(bp_eff.reshape(1, 1, 128), (128, 4, 128)).reshape(128, 512)
    )
    # q-bias for pair tiles: rows (hr, d) 0:64; col X selects the pair
    bq2 = np.zeros((64, 2), dtype=np.float32)
    bq2[:, 0] = bqs[0:64, 0]
    bq2[:, 1] = bqs[64:128, 0]
    bq2 = np.ascontiguousarray(np.pad(bq2, ((0, 64), (0, 0))))   # [128,2]
    ident = np.eye(128, dtype=np.float32)
    # mask, transposed chunk-dense: rows = chunk token (b*64+kk), cols repeat
    # over 4 chunks of (b'*64+qq); 0 iff same batch and kk<=qq else NEG
    bkk = np.arange(128).reshape(128, 1)
    bqq = np.arange(128).reshape(1, 128)
    ok = ((bkk // 64) == (bqq // 64)) & ((bkk % 64) <= (bqq % 64))
    m128 = np.where(ok, 0.0, NEG).astype(np.float32)             # [128,128]
    maskT = np.ascontiguousarray(np.tile(m128, (1, 4)))          # [128,512]
    # sums stationary: sumsI[:, 4h+j] = (j==h)
    sumsI = np.zeros((128, 16), dtype=np.float32)
    for h in range(4):
        sumsI[:, 4 * h + h] = 1.0
    # bcast stationary (fp32, used as fp32r): row h covers its head's 32
    # features; rbA uses cols 0:64 (heads 0,1), rbB cols 64:128 (heads 2,3)
    S4b = np.zeros((128, 128), dtype=np.float32)
    for h in range(4):
        c0 = 64 * (h // 2) + 32 * (h % 2)
        S4b[h, c0:c0 + 32] = 1.0

    f32_parts = [
        ("ident", ident),
        ("bq2", bq2),
        ("biasP", biasP.astype(np.float32)),
    ]
    # proj stationaries are the [64,512] yT tiles at base 0, so both Wp row
    # halves must also sit at partitions 0:64: wps[:, 128X:128(X+1)] = Wp half X
    wps = np.ascontiguousarray(
        np.pad(np.concatenate([W_proj[0:64], W_proj[64:128]], axis=1),
               ((0, 64), (0, 0)))
    )                                                            # [128,256]
    bf_parts = [
        ("wq", Wq), ("wk", Wk), ("wv", Wv),
        ("wps", wps),
        ("identb", ident),
        ("maskT", maskT),
        ("sumsI", sumsI),
        ("S4b", S4b),
    ]

    def pack(parts, npdt):
        arrs = [np.asarray(a, dtype=np.float32).astype(npdt) for _, a in parts]
        packed = np.concatenate(arrs, axis=1)
        offs, off = {}, 0
        for (name, _), a in zip(parts, arrs):
            offs[name] = (off, a.shape[1])
            off += a.shape[1]
        return np.ascontiguousarray(packed), offs

    cf, cf_offs = pack(f32_parts, np.float32)
    cb, cb_offs = pack(bf_parts, NP_BF16)
    return cf, cf_offs, cb, cb_offs


def _build_program(cf_offs, cf_cols, cb_offs, cb_cols, ntok=TOK, stage=9):
    nmega = ntok // MEGA
    nc = bacc.Bacc()
    x_p = nc.declare_dram_parameter("x", [ntok, D], F32, isOutput=False)
    y_p = nc.declare_dram_parameter("y", [ntok, D], F32, isOutput=True)
    cf_p = nc.declare_dram_parameter("cf", [128, cf_cols], F32, isOutput=False)
    cb_p = nc.declare_dram_parameter("cb", [128, cb_cols], BF16, isOutput=False)

    # token index: t = m*512 + c*128 + p  (c = chunk, p = partition = b*64+qq)
    x_v = x_p.rearrange("(m c p) d -> m p c d", c=4, p=128)
    y_v = y_p.rearrange("(m c p) d -> m p c d", c=4, p=128)

    Copy = mybir.ActivationFunctionType.Copy
    Exp = mybir.ActivationFunctionType.Exp

    with tile.TileContext(nc) as tc, ExitStack() as ctx:
        cpool = ctx.enter_context(tc.tile_pool(name="consts", bufs=1))
        sb = ctx.enter_context(tc.tile_pool(name="sb", bufs=2))
        psA = ctx.enter_context(tc.tile_pool(name="psA", bufs=4, space="PSUM"))
        psB = ctx.enter_context(tc.tile_pool(name="psB", bufs=1, space="PSUM"))

        callf = cpool.tile([128, cf_cols], F32, tag="c_f32")
        nc.sync.dma_start(out=callf[:], in_=cf_p[:])
        callb = cpool.tile([128, cb_cols], BF16, tag="c_bf")
        nc.sync.dma_start(out=callb[:], in_=cb_p[:])
        ctf = {n: callf[:, o:o + w] for n, (o, w) in cf_offs.items()}
        ctb = {n: callb[:, o:o + w] for n, (o, w) in cb_offs.items()}

        # software-pipeline state: proj+bias+DMA of mega m-1 emitted inside m
        pend = {}

        def emit_proj(p):
            yTA, yTB, m = p["yTA"], p["yTB"], p["m"]
            yf = psA.tile([128, 512], F32, tag="mm512")
            for c in range(4):
                nc.tensor.matmul(
                    yf[:, c * 128:(c + 1) * 128],
                    yTA[:, c * 128:(c + 1) * 128],
                    ctb["wps"][0:64, 0:128],
                    start=True, stop=False,
                )
                nc.tensor.matmul(
                    yf[:, c * 128:(c + 1) * 128],
                    yTB[:, c * 128:(c + 1) * 128],
                    ctb["wps"][0:64, 128:256],
                    start=False, stop=True,
                )
            y_out = sb.tile([128, 512], F32, tag="y_out")
            nc.vector.tensor_add(y_out[:], yf[:], ctf["biasP"])
            nc.sync.dma_start(
                out=y_v[m], in_=y_out[:].rearrange("p (c d) -> p c d", c=4)
            )

        for m in range(nmega):
            # ---- load x natural [p, c, d]
            x_nat = sb.tile([128, 4, 128], F32, tag="x_nat")
            nc.sync.dma_start(out=x_nat[:], in_=x_v[m])

            # ---- x^T via PE transpose (fp32 in, bf16 out through ACT copy)
            xT_ps = psA.tile([128, 512], F32, tag="mm512")
            for c in range(4):
                nc.tensor.transpose(
                    xT_ps[:, c * 128:(c + 1) * 128], x_nat[:, c, :], ctf["ident"]
                )
            xT = sb.tile([128, 512], BF16, tag="xT")
            nc.scalar.activation(xT[:], xT_ps[:], Copy)
            if stage <= 1:
                nc.sync.dma_start(
                    out=y_v[m], in_=xT_ps[:].rearrange("p (c d) -> p c d", c=4)
                )
                continue

            # ---- q^T, k^T as head-pair tiles (X = head//2), base 0
            qk_ps, qk_sb = [], []
            for X in range(2):
                qp = psA.tile([64, 512], F32, tag="mm512")
                nc.tensor.matmul(
                    qp[:], ctb["wq"][:, 64 * X:64 * (X + 1)], xT[:],
                    start=True, stop=True,
                )
                qk_ps.append(qp)
            for X in range(2):
                kp = psA.tile([64, 512], F32, tag="mm512")
                nc.tensor.matmul(
                    kp[:], ctb["wk"][:, 64 * X:64 * (X + 1)], xT[:],
                    start=True, stop=True,
                )
                qk_ps.append(kp)
            qtA = sb.tile([64, 512], BF16, tag="qtA")
            nc.vector.tensor_scalar_add(qtA[:], qk_ps[0][:], ctf["bq2"][0:64, 0:1])
            qtB = sb.tile([64, 512], BF16, tag="qtB")
            nc.vector.tensor_scalar_add(qtB[:], qk_ps[1][:], ctf["bq2"][0:64, 1:2])
            ktA = sb.tile([64, 512], BF16, tag="ktA")
            nc.vector.tensor_copy(ktA[:], qk_ps[2][:])
            ktB = sb.tile([64, 512], BF16, tag="ktB")
            nc.vector.tensor_copy(ktB[:], qk_ps[3][:])

            # ---- proj of previous mega (PE fill while DVE normalizes)
            if pend:
                emit_proj(pend)
                pend = {}

            # ---- v natural [tok, feat]
            v_ps = psA.tile([128, 512], F32, tag="mm512")
            for c in range(4):
                nc.tensor.matmul(
                    v_ps[:, c * 128:(c + 1) * 128],
                    xT[:, c * 128:(c + 1) * 128],
                    ctb["wv"],
                    start=True, stop=True,
                )
            v_s = sb.tile([128, 512], BF16, tag="v_s")
            nc.scalar.activation(v_s[:], v_ps[:], Copy)

            # ---- scores, transposed chunk-dense; PSUM bank = head
            # sc col = 512*h + 128*c + (b*64+qq)
            sc = psB.tile([128, 2048], F32, tag="sc")
            attn_u = sb.tile([128, 2048], BF16, tag="attn")
            for h in range(4):
                qt = (qtA, qtB)[h // 2]
                kt = (ktA, ktB)[h // 2]
                r0 = 32 * (h % 2)
                nc.tensor.matmul(
                    sc[:, 512 * h:512 * (h + 1)],
                    ctb["identb"], ctb["maskT"],
                    start=True, stop=False, skip_group_check=True,
                )
                for c in range(4):
                    nc.tensor.matmul(
                        sc[:, 512 * h + 128 * c:512 * h + 128 * (c + 1)],
                        kt[r0:r0 + 32, 128 * c:128 * (c + 1)],
                        qt[r0:r0 + 32, 128 * c:128 * (c + 1)],
                        start=False, stop=(c == 3), skip_group_check=True,
                    )
                # exp per bank so downstream consumers never wait on one big op
                nc.scalar.activation(
                    attn_u[:, 512 * h:512 * (h + 1)],
                    sc[:, 512 * h:512 * (h + 1)], Exp,
                )
            if stage <= 3:
                nc.sync.dma_start(
                    out=y_v[m], in_=attn_u[:, 0:512].rearrange("p (c d) -> p c d", c=4)
                )
                continue

            # ---- denominators: su[h, (c, b, qq)] via 4 accumulating matmuls
            su = psA.tile([4, 512], F32, tag="mm512")
            for h in range(4):
                nc.tensor.matmul(
                    su[:],
                    ctb["sumsI"][:, 4 * h:4 * (h + 1)],
                    attn_u[:, 512 * h:512 * (h + 1)],
                    start=(h == 0), stop=(h == 3),
                )
            rec = sb.tile([4, 512], F32, tag="rec")
            nc.vector.reciprocal_approx_fast(out=rec[:], in_=su[:])
            rec_b = sb.tile([4, 512], BF16, tag="rec_b")
            nc.gpsimd.tensor_copy(rec_b[:], rec[:])

            # ---- attn @ v, unnormalized -> ytX[32*(h%2)+d, 128c + b*64+qq]
            yts = []
            for X in range(2):
                yt = psA.tile([64, 512], F32, tag="mm512")
                yts.append(yt)
                for hr in range(2):
                    h = 2 * X + hr
                    for c in range(4):
                        nc.tensor.matmul(
                            yt[32 * hr:32 * (hr + 1), 128 * c:128 * (c + 1)],
                            v_s[:, 128 * c + 32 * h:128 * c + 32 * (h + 1)],
                            attn_u[:, 512 * h + 128 * c:512 * h + 128 * (c + 1)],
                            start=True, stop=True,
                        )
            if stage <= 5:
                nc.sync.dma_start(
                    out=y_v[m], in_=yts[0][:].rearrange("p (c d) -> p c d", c=4)
                )
                continue

            # ---- rec broadcast over head features (fp32r full-rate matmuls)
            rbs = []
            for X in range(2):
                rb = psA.tile([64, 512], F32, tag="mm512")
                rbs.append(rb)
                nc.tensor.matmul(
                    rb[:],
                    ctb["S4b"][0:4, 64 * X:64 * (X + 1)],
                    rec_b[:],
                    start=True, stop=True,
                )

            # ---- normalize: rb -> SBUF (ACT), then one-PSUM DVE muls
            rbsS = []
            for X in range(2):
                rbS = sb.tile([64, 512], BF16, tag=f"rbS{X}")
                nc.scalar.activation(rbS[:], rbs[X][:], Copy)
                rbsS.append(rbS)
            yTA = sb.tile([64, 512], BF16, tag="yTA")
            nc.vector.tensor_mul(yTA[:], yts[0][:], rbsS[0][:])
            yTB = sb.tile([64, 512], BF16, tag="yTB")
            nc.vector.tensor_mul(yTB[:], yts[1][:], rbsS[1][:])

            pend = {"yTA": yTA, "yTB": yTB, "m": m}

        if pend:
            emit_proj(pend)
            pend = {}
    nc.compile()
    return nc


def kernel(x, k_in, v_in, W_attn, b_attn, W_proj, b_proj):
    x = np.asarray(x, dtype=np.float32)
    cf, cf_offs, cb, cb_offs = _host_consts(
        np.asarray(W_attn, dtype=np.float32),
        np.asarray(b_attn, dtype=np.float32),
        np.asarray(W_proj, dtype=np.float32),
        np.asarray(b_proj, dtype=np.float32),
    )
    key = "prog"
    if key not in _CACHE:
        _CACHE[key] = _build_program(cf_offs, cf.shape[1], cb_offs, cb.shape[1])
    nc = _CACHE[key]

    in_maps = []
    for i in range(NCORES):
        shard = np.ascontiguousarray(x[i * BC:(i + 1) * BC].reshape(TOK, D))
        in_maps.append({"x": shard, "cf": cf, "cb": cb})

    res = run_bass_kernel_spmd(nc, in_maps, list(range(NCORES)))
    global LAST_RESULT
    LAST_RESULT = res
    outs = [res.results[i]["y"].reshape(BC, T, D) for i in range(NCORES)]
    return np.concatenate(outs, axis=0)


if __name__ == "__main__":
    rng = np.random.default_rng(0)
    xs = rng.standard_normal((B, T, D), dtype=np.float32)
    Wa = rng.standard_normal((D, 3 * D), dtype=np.float32) / np.sqrt(D)
    ba = rng.standard_normal(3 * D, dtype=np.float32) * 0.01
    Wp = rng.standard_normal((D, D), dtype=np.float32) / np.sqrt(D)
    bp = rng.standard_normal(D, dtype=np.float32) * 0.01
    out = kernel(xs, None, None, Wa, ba, Wp, bp)
    print(out.shape, out.dtype)


# revision 14
# speedup vs baseline: 1.9120x; 1.1415x over previous
"""Causal self-attention kernel for Trainium2, 8 NeuronCores, data-parallel over batch.

Problem: B=4096 independent attentions, T=64, DIM=128, 4 heads of 32; y = proj(attn(x)).
k_in / v_in inputs are unused by the module (overwritten internally) -> never shipped.

v1 "chunk-dense bf16" design (per core: 512 batches = 32768 tokens, 64 mega-tiles
of 512 tokens = 4 chunks of 128 tokens = 2 batches each):
  - All matmuls bf16 (fp32 matmuls run as 2 half-speed passes; bf16 is 4x).
  - x loaded natural, PE-transposed (fp32), cast to bf16 in the PSUM->SBUF copy.
  - q^T/k^T as head-pair tiles [64,512] (partition base 96 is rejected by the
    toolchain, so features of head 3 must sit at base 32 of a pair tile).
    q-bias added in the PSUM->SBUF copy (per-partition tensor_scalar); k-bias
    dropped (softmax invariant); v-bias folded into proj bias on host.
  - scores chunk-dense TRANSPOSED: per (head h, chunk c) one [32,128]x[32,128]
    -> [128,128] matmul; the 2 batches inside a chunk are separated by the mask
    seed (-80 on cross-batch and causal-invalid pairs), so exp() kills them.
    PSUM bank = head; each bank's accumulation group is (seed + 4 scorers).
  - softmax denominators: 4 accumulating indicator matmuls -> su[4,512]
    (row = head), reciprocal_approx_fast (18 bits, ~5x faster than
    nc.vector.reciprocal), broadcast over each head's 32 features with fp32r
    matmuls (fp32r runs 1 cyc/row when out free >= 256 -- no bf16 cast needed).
  - attn@v with UNNORMALIZED attn (normalization commutes with the key-sum):
    per (h, c) one [128,32]x[128,128] matmul -> ytX[32*(h%2)+d, tok] for
    X=h//2; each [64,512] ytX is normalized by rec in the PSUM->SBUF copy.
  - proj natural: per chunk two accumulating K=64 matmuls (yTA/yTB stationary),
    bias via gpsimd tensor_add; proj of mega m-1 is emitted inside mega m
    (software pipelining) so the PE never waits on the DVE normalize.
  - elementwise ops spread across ACT (xT copy, exp), DVE (q-bias, recip,
    normalize), Pool/GpSimd (k/v copies, y bias add) to keep all engines busy.
"""

import sys

for _p in ("/opt/trn_rl_repo", "/root/.axon_site/_ro/trn_rl_repo"):
    if _p not in sys.path:
        sys.path.insert(0, _p)

from contextlib import ExitStack

import numpy as np
import ml_dtypes

import concourse.bass as bass
import concourse.tile as tile
from concourse import bacc
from concourse import mybir
from concourse.bass_utils import run_bass_kernel_spmd

F32 = mybir.dt.float32
F32R = mybir.dt.float32r
BF16 = mybir.dt.bfloat16
NP_BF16 = ml_dtypes.bfloat16

B, T, D, H, HS = 4096, 64, 128, 4, 32
NCORES = 8
BC = B // NCORES            # 512 batches per core
TOK = BC * T                # 32768 tokens per core
MEGA = 512                  # tokens per mega-tile (4 chunks of 128 = 8 batches)
NMEGA = TOK // MEGA         # 64
SCALE = 1.0 / float(np.sqrt(HS))
NEG = -80.0                 # additive causal mask (exp(-80) ~ 1.8e-35, harmless)

_CACHE = {}
LAST_RESULT = None


def _host_consts(W_attn, b_attn, W_proj, b_proj):
    """Precompute constant tiles on host: one fp32 pack and one bf16 pack."""
    Wq = np.ascontiguousarray(W_attn[:, 0:128] * SCALE)          # [128,128]
    Wk = np.ascontiguousarray(W_attn[:, 128:256])
    Wv = np.ascontiguousarray(W_attn[:, 256:384])
    bqs = (b_attn[0:128] * SCALE).reshape(128, 1)                # [128,1]
    bv = b_attn[256:384]
    bp_eff = b_proj + bv @ W_proj                                # [128]
    biasP = np.ascontiguousarray(
        np.broadcast_to(bp_eff.reshape(1, 1, 128), (128, 4, 128)).reshape(128, 512)
    )
    # q-bias for pair tiles: rows (hr, d) 0:64; col X selects the pair
    bq2 = np.zeros((64, 2), dtype=np.float32)
    bq2[:, 0] = bqs[0:64, 0]
    bq2[:, 1] = bqs[64:128, 0]
    bq2 = np.ascontiguousarray(np.pad(bq2, ((0, 64), (0, 0))))   # [128,2]
    ident = np.eye(128, dtype=np.float32)
    # mask, transposed chunk-dense: rows = chunk token (b*64+kk), cols repeat
    # over 4 chunks of (b'*64+qq); 0 iff same batch and kk<=qq else NEG
    bkk = np.arange(128).reshape(128, 1)
    bqq = np.arange(128).reshape(1, 128)
    ok = ((bkk // 64) == (bqq // 64)) & ((bkk % 64) <= (bqq % 64))
    m128 = np.where(ok, 0.0, NEG).astype(np.float32)             # [128,128]
    maskT = np.ascontiguousarray(np.tile(m128, (1, 4)))          # [128,512]
    # sums stationary: sumsI[:, 4h+j] = (j==h)
    sumsI = np.zeros((128, 16), dtype=np.float32)
    for h in range(4):
        sumsI[:, 4 * h + h] = 1.0
    # bcast stationary (fp32, used as fp32r): row h covers its head's 32
    # features; rbA uses cols 0:64 (heads 0,1), rbB cols 64:128 (heads 2,3)
    S4b = np.zeros((128, 128), dtype=np.float32)
    for h in range(4):
        c0 = 64 * (h // 2) + 32 * (h % 2)
        S4b[h, c0:c0 + 32] = 1.0

    f32_parts = [
        ("ident", ident),
        ("bq2", bq2),
        ("biasP", biasP.astype(np.float32)),
    ]
    # proj stationaries are the [64,512] yT tiles at base 0, so both Wp row
    # halves must also sit at partitions 0:64: wps[:, 128X:128(X+1)] = Wp half X
    wps = np.ascontiguousarray(
        np.pad(np.concatenate([W_proj[0:64], W_proj[64:128]], axis=1),
               ((0, 64), (0, 0)))
    )                                                            # [128,256]
    bf_parts = [
        ("wq", Wq), ("wk", Wk), ("wv", Wv),
        ("wps", wps),
        ("identb", ident),
        ("maskT", maskT),
        ("sumsI", sumsI),
        ("S4b", S4b),
    ]

    def pack(parts, npdt):
        arrs = [np.asarray(a, dtype=np.float32).astype(npdt) for _, a in parts]
        packed = np.concatenate(arrs, axis=1)
        offs, off = {}, 0
        for (name, _), a in zip(parts, arrs):
            offs[name] = (off, a.shape[1])
            off += a.shape[1]
        return np.ascontiguousarray(packed), offs

    cf, cf_offs = pack(f32_parts, np.float32)
    cb, cb_offs = pack(bf_parts, NP_BF16)
    return cf, cf_offs, cb, cb_offs


def _build_program(cf_offs, cf_cols, cb_offs, cb_cols, ntok=TOK, stage=9):
    nmega = ntok // MEGA
    nc = bacc.Bacc()
    x_p = nc.declare_dram_parameter("x", [ntok, D], F32, isOutput=False)
    y_p = nc.declare_dram_parameter("y", [ntok, D], F32, isOutput=True)
    cf_p = nc.declare_dram_parameter("cf", [128, cf_cols], F32, isOutput=False)
    cb_p = nc.declare_dram_parameter("cb", [128, cb_cols], BF16, isOutput=False)

    # token index: t = m*512 + c*128 + p  (c = chunk, p = partition = b*64+qq)
    x_v = x_p.rearrange("(m c p) d -> m p c d", c=4, p=128)
    y_v = y_p.rearrange("(m c p) d -> m p c d", c=4, p=128)

    Copy = mybir.ActivationFunctionType.Copy
    Exp = mybir.ActivationFunctionType.Exp

    with tile.TileContext(nc) as tc, ExitStack() as ctx:
        cpool = ctx.enter_context(tc.tile_pool(name="consts", bufs=1))
        sb = ctx.enter_context(tc.tile_pool(name="sb", bufs=2))
        psA = ctx.enter_context(tc.tile_pool(name="psA", bufs=4, space="PSUM"))
        psB = ctx.enter_context(tc.tile_pool(name="psB", bufs=1, space="PSUM"))

        callf = cpool.tile([128, cf_cols], F32, tag="c_f32")
        nc.sync.dma_start(out=callf[:], in_=cf_p[:])
        callb = cpool.tile([128, cb_cols], BF16, tag="c_bf")
        nc.sync.dma_start(out=callb[:], in_=cb_p[:])
        ctf = {n: callf[:, o:o + w] for n, (o, w) in cf_offs.items()}
        ctb = {n: callb[:, o:o + w] for n, (o, w) in cb_offs.items()}

        # persistent zero-padded q tiles for K=64 pair-packed scorers:
        # qt2[ph][X][(hr', d) row, (c, hr, bqq) col]; the hr'!=hr blocks stay 0
        # (written once here), so the pair contraction picks out head 2X+hr.
        qt2 = [[cpool.tile([64, 4, 2, 128], BF16, tag=f"qt2_{ph}{X}",
                           name=f"qt2_{ph}{X}")
                for X in range(2)] for ph in range(2)]
        for ph in range(2):
            for X in range(2):
                nc.vector.memset(qt2[ph][X][0:32, :, 1, :], 0.0)
                nc.vector.memset(qt2[ph][X][32:64, :, 0, :], 0.0)

        # software-pipeline state: proj+bias+DMA of mega m-1 emitted inside m
        pend = {}

        def emit_proj(p):
            yTA, yTB, m = p["yTA"], p["yTB"], p["m"]
            yf = psA.tile([128, 512], F32, tag="mm512")
            for c in range(4):
                nc.tensor.matmul(
                    yf[:, c * 128:(c + 1) * 128],
                    yTA[:, c * 128:(c + 1) * 128],
                    ctb["wps"][0:64, 0:128],
                    start=True, stop=False,
                )
                nc.tensor.matmul(
                    yf[:, c * 128:(c + 1) * 128],
                    yTB[:, c * 128:(c + 1) * 128],
                    ctb["wps"][0:64, 128:256],
                    start=False, stop=True,
                )
            y_out = sb.tile([128, 512], F32, tag="y_out")
            nc.vector.tensor_add(y_out[:], yf[:], ctf["biasP"])
            nc.sync.dma_start(
                out=y_v[m], in_=y_out[:].rearrange("p (c d) -> p c d", c=4)
            )

        for m in range(nmega):
            # ---- load x natural [p, c, d]
            x_nat = sb.tile([128, 4, 128], F32, tag="x_nat")
            nc.sync.dma_start(out=x_nat[:], in_=x_v[m])

            # ---- x^T via PE transpose (fp32 in, bf16 out through ACT copy)
            xT_ps = psA.tile([128, 512], F32, tag="mm512")
            for c in range(4):
                nc.tensor.transpose(
                    xT_ps[:, c * 128:(c + 1) * 128], x_nat[:, c, :], ctf["ident"]
                )
            xT = sb.tile([128, 512], BF16, tag="xT")
            nc.scalar.activation(xT[:], xT_ps[:], Copy)
            if stage <= 1:
                nc.sync.dma_start(
                    out=y_v[m], in_=xT_ps[:].rearrange("p (c d) -> p c d", c=4)
                )
                continue

            # ---- q^T, k^T as head-pair tiles (X = head//2), base 0
            qk_ps, qk_sb = [], []
            for X in range(2):
                qp = psA.tile([64, 512], F32, tag="mm512")
                nc.tensor.matmul(
                    qp[:], ctb["wq"][:, 64 * X:64 * (X + 1)], xT[:],
                    start=True, stop=True,
                )
                qk_ps.append(qp)
            for X in range(2):
                kp = psA.tile([64, 512], F32, tag="mm512")
                nc.tensor.matmul(
                    kp[:], ctb["wk"][:, 64 * X:64 * (X + 1)], xT[:],
                    start=True, stop=True,
                )
                qk_ps.append(kp)
            for X in range(2):
                q2 = qt2[m % 2][X]
                qp = qk_ps[X][:].rearrange("p (c q) -> p c q", c=4)
                nc.vector.tensor_scalar_add(
                    q2[0:32, :, 0, :], qp[0:32], ctf["bq2"][0:32, X:X + 1])
                nc.vector.tensor_scalar_add(
                    q2[32:64, :, 1, :], qp[32:64], ctf["bq2"][32:64, X:X + 1])
            ktA = sb.tile([64, 512], BF16, tag="ktA")
            nc.scalar.activation(ktA[:], qk_ps[2][:], Copy)
            ktB = sb.tile([64, 512], BF16, tag="ktB")
            nc.scalar.activation(ktB[:], qk_ps[3][:], Copy)

            # ---- proj of previous mega (PE fill while DVE normalizes)
            if pend:
                emit_proj(pend)
                pend = {}

            # ---- v natural [tok, feat]
            v_ps = psA.tile([128, 512], F32, tag="mm512")
            for c in range(4):
                nc.tensor.matmul(
                    v_ps[:, c * 128:(c + 1) * 128],
                    xT[:, c * 128:(c + 1) * 128],
                    ctb["wv"],
                    start=True, stop=True,
                )
            v_s = sb.tile([128, 512], BF16, tag="v_s")
            nc.scalar.activation(v_s[:], v_ps[:], Copy)

            # ---- scores, transposed chunk-dense, K=64 pair-packed
            # sc col = 1024*X + 256*c + 128*hr + (b*64+qq); bank = (X, c-pair)
            # mask seeded by DVE/ACT psum writes (off the PE), scorers
            # accumulate on top with start=False
            sc = psB.tile([128, 2048], F32, tag="sc")
            attn_u = sb.tile([128, 2048], BF16, tag="attn")
            for X in range(2):
                kt = (ktA, ktB)[X]
                for cp in range(2):
                    # bank (X, cp): seed the mask, then 2 pair-packed scorers
                    nc.tensor.matmul(
                        sc[:, 1024 * X + 512 * cp:1024 * X + 512 * (cp + 1)],
                        ctb["identb"], ctb["maskT"],
                        start=True, stop=False, skip_group_check=True,
                    )
                    for cl in range(2):
                        c = 2 * cp + cl
                        nc.tensor.matmul(
                            sc[:, 1024 * X + 256 * c:1024 * X + 256 * (c + 1)],
                            kt[:, 128 * c:128 * (c + 1)],
                            qt2[m % 2][X][:, c, :, :],
                            start=False, stop=(cl == 1), skip_group_check=True,
                        )
                # exp per X-half once its 4 scorers are done
                nc.scalar.activation(
                    attn_u[:, 1024 * X:1024 * (X + 1)],
                    sc[:, 1024 * X:1024 * (X + 1)], Exp,
                )
            if stage <= 3:
                nc.sync.dma_start(
                    out=y_v[m], in_=attn_u[:, 0:512].rearrange("p (c d) -> p c d", c=4)
                )
                continue

            # ---- denominators: su[h, (c, b, qq)] via 4 accumulating matmuls
            su = psA.tile([4, 512], F32, tag="mm512")
            attn_v4 = attn_u[:].rearrange(
                "p (x c h q) -> p x c h q", x=2, c=4, h=2)
            for h in range(4):
                nc.tensor.matmul(
                    su[:],
                    ctb["sumsI"][:, 4 * h:4 * (h + 1)],
                    attn_v4[:, h // 2, :, h % 2, :],
                    start=(h == 0), stop=(h == 3),
                )
            rec = sb.tile([4, 512], F32, tag="rec")
            nc.vector.reciprocal_approx_fast(out=rec[:], in_=su[:])
            rec_b = sb.tile([4, 512], BF16, tag="rec_b")
            nc.vector.tensor_copy(rec_b[:], rec[:])

            # ---- attn @ v, unnormalized -> ytX[32*(h%2)+d, 128c + b*64+qq]
            yts = []
            for X in range(2):
                yt = psA.tile([64, 512], F32, tag="mm512")
                yts.append(yt)
                for hr in range(2):
                    h = 2 * X + hr
                    for c in range(4):
                        nc.tensor.matmul(
                            yt[32 * hr:32 * (hr + 1), 128 * c:128 * (c + 1)],
                            v_s[:, 128 * c + 32 * h:128 * c + 32 * (h + 1)],
                            attn_u[:, 1024 * X + 256 * c + 128 * hr:
                                   1024 * X + 256 * c + 128 * (hr + 1)],
                            start=True, stop=True,
                        )
            if stage <= 5:
                nc.sync.dma_start(
                    out=y_v[m], in_=yts[0][:].rearrange("p (c d) -> p c d", c=4)
                )
                continue

            # ---- rec broadcast over head features (fp32r full-rate matmuls)
            rbs = []
            for X in range(2):
                rb = psA.tile([64, 512], F32, tag="mm512")
                rbs.append(rb)
                nc.tensor.matmul(
                    rb[:],
                    ctb["S4b"][0:4, 64 * X:64 * (X + 1)],
                    rec_b[:],
                    start=True, stop=True,
                )

            # ---- normalize: rb -> SBUF (ACT), then one-PSUM DVE muls
            rbsS = []
            for X in range(2):
                rbS = sb.tile([64, 512], BF16, tag=f"rbS{X}")
                nc.scalar.activation(rbS[:], rbs[X][:], Copy)
                rbsS.append(rbS)
            yTA = sb.tile([64, 512], BF16, tag="yTA")
            nc.vector.tensor_mul(yTA[:], yts[0][:], rbsS[0][:])
            yTB = sb.tile([64, 512], BF16, tag="yTB")
            nc.vector.tensor_mul(yTB[:], yts[1][:], rbsS[1][:])

            pend = {"yTA": yTA, "yTB": yTB, "m": m}

        if pend:
            emit_proj(pend)
            pend = {}
    nc.compile()
    return nc


def kernel(x, k_in, v_in, W_attn, b_attn, W_proj, b_proj):
    x = np.asarray(x, dtype=np.float32)
    cf, cf_offs, cb, cb_offs = _host_consts(
        np.asarray(W_attn, dtype=np.float32),
        np.asarray(b_attn, dtype=np.float32),
        np.asarray(W_proj, dtype=np.float32),
        np.asarray(b_proj, dtype=np.float32),
    )
    key = "prog"
    if key not in _CACHE:
        _CACHE[key] = _build_program(cf_offs, cf.shape[1], cb_offs, cb.shape[1])
    nc = _CACHE[key]

    in_maps = []
    for i in range(NCORES):
        shard = np.ascontiguousarray(x[i * BC:(i + 1) * BC].reshape(TOK, D))
        in_maps.append({"x": shard, "cf": cf, "cb": cb})

    res = run_bass_kernel_spmd(nc, in_maps, list(range(NCORES)))
    global LAST_RESULT
    LAST_RESULT = res
    outs = [res.results[i]["y"].reshape(BC, T, D) for i in range(NCORES)]
    return np.concatenate(outs, axis=0)


if __name__ == "__main__":
    rng = np.random.default_rng(0)
    xs = rng.standard_normal((B, T, D), dtype=np.float32)
    Wa = rng.standard_normal((D, 3 * D), dtype=np.float32) / np.sqrt(D)
    ba = rng.standard_normal(3 * D, dtype=np.float32) * 0.01
    Wp = rng.standard_normal((D, D), dtype=np.float32) / np.sqrt(D)
    bp = rng.standard_normal(D, dtype=np.float32) * 0.01
    out = kernel(xs, None, None, Wa, ba, Wp, bp)
    print(out.shape, out.dtype)
